# revision 5
# baseline (speedup 1.0000x reference)
"""Top-1 MoE layer (B=2, S=2048, D=1024, E=8, H=4096) on 8 TRN2 NeuronCores.

Strategy: expert parallelism. The gate (a [T,8] logit matmul, ~0.1% of the
model's FLOPs) plus top-1 routing runs on host; tokens are gathered per
expert ("all-to-all by top-1 expert id") and each NeuronCore runs one
expert's MLP over its tokens, padded to a fixed capacity C. Device compute
is bf16 matmuls with fp32 PSUM accumulation and exact-erf GELU on the
scalar engine. Outputs are scattered back to token order on host; the aux
loss (softmax statistics) is computed on host.
"""

import numpy as np
import ml_dtypes

import concourse.bass as bass
import concourse.mybir as mybir
import concourse.tile as tile
from concourse import bass_utils

B, S, D, E = 2, 2048, 1024, 8
H = 4 * D
T = B * S
C = 640          # per-expert token capacity (binomial max for T=4096,E=8 is ~583)
NH = 2           # token tiles per core
CT = C // NH     # tokens per matmul (psum free dim, <=512)
KD = D // 128    # k tiles for matmul1
MH = H // 128    # m tiles for matmul1 / k tiles for matmul2
MD = D // 128    # m tiles for matmul2

_BF16 = mybir.dt.bfloat16
_F32 = mybir.dt.float32


def _legalize_waits(nc):
    """Split multi-wait instructions: this walrus accepts at most 1 wait
    condition per instruction (2 for EventSemaphore). Excess waits are
    hoisted onto preceding NoOps on the same engine, which blocks the
    sequencer on each condition in turn before the original instruction."""
    for func in nc.m.functions:
        for block in func.blocks:
            insts = list(block.instructions)
            out = []
            changed = False
            for inst in insts:
                si = inst.sync_info
                cap = 2 if inst.opcode == "EventSemaphore" else 1
                if si is not None and len(si.on_wait) > cap:
                    waits = list(si.on_wait)
                    keep, hoist = waits[-cap:], waits[:-cap]
                    for i, w in enumerate(hoist):
                        nop = mybir.InstNoOp(name=f"{inst.name}_wsplit{i}")
                        nop.engine = inst.engine
                        nop.sync_info = mybir.SyncInfo(on_wait=[w], on_update=[])
                        out.append(nop)
                    inst.sync_info = mybir.SyncInfo(
                        on_wait=keep, on_update=list(si.on_update)
                    )
                    changed = True
                out.append(inst)
            if changed:
                block.instructions = out


def build_moe_kernel():
    nc = bass.Bass(name="moe_expert", enable_partition_id=False)
    xt_d = nc.dram_tensor("xt", [128, KD, C], _BF16, kind="ExternalInput")
    w1_d = nc.dram_tensor("w1t", [MH, 128, KD * 128], _BF16, kind="ExternalInput")
    w2_d = nc.dram_tensor("w2t", [MD, 128, MH * 128], _BF16, kind="ExternalInput")
    b1_d = nc.dram_tensor("b1t", [128, MH], _F32, kind="ExternalInput")
    b2_d = nc.dram_tensor("b2t", [128, MD], _F32, kind="ExternalInput")
    yt_d = nc.dram_tensor("yt", [MD, 128, C], _F32, kind="ExternalOutput")

    with tile.TileContext(nc) as tc:
        with (
            tc.tile_pool(name="xp", bufs=1) as xpool,
            tc.tile_pool(name="bp", bufs=1) as bpool,
            tc.tile_pool(name="w1p", bufs=3) as w1pool,
            tc.tile_pool(name="hp", bufs=1) as hpool,
            tc.tile_pool(name="w2p", bufs=3) as w2pool,
            tc.tile_pool(name="op", bufs=3) as opool,
            tc.tile_pool(name="ps", bufs=8, space="PSUM") as pspool,
        ):
            xt = xpool.tile([128, KD, C], _BF16)
            nc.sync.dma_start(xt[:], xt_d[:])
            b1 = bpool.tile([128, MH], _F32, tag="b1")
            b2 = bpool.tile([128, MD], _F32, tag="b2")
            nc.sync.dma_start(b1[:], b1_d[:])
            nc.sync.dma_start(b2[:], b2_d[:])

            ht = hpool.tile([128, MH, C], _BF16)

            # phase 1: ht[:, mt, :] = gelu(w1[:, :, mt].T @ x + b1[mt])
            for mt in range(MH):
                w1t = w1pool.tile([128, KD * 128], _BF16)
                nc.sync.dma_start(w1t[:], w1_d[mt])
                for nh in range(NH):
                    ps = pspool.tile([128, CT], _F32)
                    for kt in range(KD):
                        nc.tensor.matmul(
                            ps[:],
                            w1t[:, kt * 128:(kt + 1) * 128],
                            xt[:, kt, nh * CT:(nh + 1) * CT],
                            start=(kt == 0),
                            stop=(kt == KD - 1),
                        )
                    nc.scalar.activation(
                        ht[:, mt, nh * CT:(nh + 1) * CT],
                        ps[:],
                        mybir.ActivationFunctionType.Gelu,
                        bias=b1[:, mt:mt + 1],
                        scale=1.0,
                    )

            # phase 2: yt[mt] = w2[:, :, mt].T @ ht + b2[mt]
            for mt in range(MD):
                w2t = w2pool.tile([128, MH * 128], _BF16)
                nc.sync.dma_start(w2t[:], w2_d[mt])
                ot = opool.tile([128, C], _F32)
                for nh in range(NH):
                    ps = pspool.tile([128, CT], _F32)
                    for kt in range(MH):
                        nc.tensor.matmul(
                            ps[:],
                            w2t[:, kt * 128:(kt + 1) * 128],
                            ht[:, kt, nh * CT:(nh + 1) * CT],
                            start=(kt == 0),
                            stop=(kt == MH - 1),
                        )
                    nc.scalar.activation(
                        ot[:, nh * CT:(nh + 1) * CT],
                        ps[:],
                        mybir.ActivationFunctionType.Identity,
                        bias=b2[:, mt:mt + 1],
                        scale=1.0,
                    )
                nc.sync.dma_start(yt_d[mt], ot[:])

    _legalize_waits(nc)
    return nc


_nc_cache = {}


def _get_nc():
    if "nc" not in _nc_cache:
        _nc_cache["nc"] = build_moe_kernel()
    return _nc_cache["nc"]


def _prepare(x, gate_w, gate_b, w1, b1, w2, b2):
    """Host-side routing: gate, top-1 argmax, aux loss, per-expert gather.

    Returns (in_maps, positions, spill, aux)."""
    x = np.asarray(x, np.float32)
    gate_w = np.asarray(gate_w, np.float32)
    gate_b = np.asarray(gate_b, np.float32)
    w1 = np.asarray(w1, np.float32)
    b1 = np.asarray(b1, np.float32)
    w2 = np.asarray(w2, np.float32)
    b2 = np.asarray(b2, np.float32)

    xf = x.reshape(T, D)
    logits = xf @ gate_w + gate_b                      # [T, E]
    top = np.argmax(logits, axis=-1)                   # [T]

    # aux loss, matching jax.nn.softmax -> mean over batch -> sum(m*m)*E
    l64 = logits.astype(np.float64)
    l64 -= l64.max(axis=-1, keepdims=True)
    p = np.exp(l64)
    p /= p.sum(axis=-1, keepdims=True)
    m = p.reshape(B, S, E).mean(axis=0)
    aux = np.float32((m * m).sum() * E)

    positions = [np.nonzero(top == e)[0] for e in range(E)]
    spill = []                                          # (expert, positions) overflow
    in_maps = []
    for e in range(E):
        pos = positions[e]
        if len(pos) > C:
            spill.append((e, pos[C:]))
            pos = pos[:C]
            positions[e] = pos
        xg = np.zeros((C, D), np.float32)
        xg[: len(pos)] = xf[pos]
        xt = np.ascontiguousarray(
            xg.reshape(C, KD, 128).transpose(2, 1, 0)
        ).astype(ml_dtypes.bfloat16)                    # [128, KD, C]
        w1t = np.ascontiguousarray(
            w1[e].reshape(KD, 128, MH, 128).transpose(2, 1, 0, 3)
        ).reshape(MH, 128, KD * 128).astype(ml_dtypes.bfloat16)
        w2t = np.ascontiguousarray(
            w2[e].reshape(MH, 128, MD, 128).transpose(2, 1, 0, 3)
        ).reshape(MD, 128, MH * 128).astype(ml_dtypes.bfloat16)
        b1t = np.ascontiguousarray(b1[e].reshape(MH, 128).T)   # [128, MH]
        b2t = np.ascontiguousarray(b2[e].reshape(MD, 128).T)   # [128, MD]
        in_maps.append({"xt": xt, "w1t": w1t, "w2t": w2t, "b1t": b1t, "b2t": b2t})

    return in_maps, positions, spill, aux


def kernel(x, gate_w, gate_b, w1, b1, w2, b2):
    in_maps, positions, spill, aux = _prepare(
        x, gate_w, gate_b, w1, b1, w2, b2
    )
    x = np.asarray(x, np.float32)
    w1 = np.asarray(w1, np.float32)
    b1 = np.asarray(b1, np.float32)
    w2 = np.asarray(w2, np.float32)
    b2 = np.asarray(b2, np.float32)
    xf = x.reshape(T, D)

    nc = _get_nc()
    res = bass_utils.run_bass_kernel_spmd(nc, in_maps, core_ids=list(range(8)))

    y = np.zeros((T, D), np.float32)
    for e in range(E):
        pos = positions[e]
        yt = res.results[e]["yt"]                       # [MD, 128, C]
        ye = yt.transpose(2, 0, 1).reshape(C, D)
        y[pos] = ye[: len(pos)]

    # overflow safety net (never triggers for the fixed problem inputs)
    if spill:
        import math
        erf = np.frompyfunc(math.erf, 1, 1)
        for e, pos in spill:
            h = xf[pos] @ w1[e] + b1[e]
            h = 0.5 * h * (1.0 + erf(h / np.sqrt(2.0)).astype(np.float32))
            y[pos] = h @ w2[e] + b2[e]

    return y.reshape(B, S, D), np.array(aux, dtype=np.float32)


# revision 7
# speedup vs baseline: 29.3073x; 29.3073x over previous
"""Top-1 MoE layer (B=2, S=2048, D=1024, E=8, H=4096) on 8 TRN2 NeuronCores.

Strategy: expert parallelism. The gate (a [T,8] logit matmul, ~0.1% of the
model's FLOPs) plus top-1 routing runs on host; tokens are gathered per
expert ("all-to-all by top-1 expert id") and each NeuronCore runs one
expert's MLP over its tokens, padded to a fixed capacity C. Device compute
is bf16 matmuls with fp32 PSUM accumulation and exact-erf GELU on the
scalar engine. Outputs are scattered back to token order on host; the aux
loss (softmax statistics) is computed on host.
"""

import numpy as np
import ml_dtypes

import concourse.bass as bass
import concourse.mybir as mybir
import concourse.tile as tile
from concourse import bass_utils

B, S, D, E = 2, 2048, 1024, 8
H = 4 * D
T = B * S
C = 640          # per-expert token capacity (binomial max for T=4096,E=8 is ~583)
NH = 2           # token tiles per core
CT = C // NH     # tokens per matmul (psum free dim, <=512)
KD = D // 128    # k tiles for matmul1
MH = H // 128    # m tiles for matmul1 / k tiles for matmul2
MD = D // 128    # m tiles for matmul2

_BF16 = mybir.dt.bfloat16
_F32 = mybir.dt.float32


def _legalize_waits(nc):
    """Split multi-wait instructions: this walrus accepts at most 1 wait
    condition per instruction (2 for EventSemaphore). Excess waits are
    hoisted onto preceding NoOps on the same engine, which blocks the
    sequencer on each condition in turn before the original instruction."""
    for func in nc.m.functions:
        for block in func.blocks:
            insts = list(block.instructions)
            out = []
            changed = False
            for inst in insts:
                si = inst.sync_info
                cap = 2 if inst.opcode == "EventSemaphore" else 1
                if si is not None and len(si.on_wait) > cap:
                    waits = list(si.on_wait)
                    keep, hoist = waits[-cap:], waits[:-cap]
                    for i, w in enumerate(hoist):
                        nop = mybir.InstNoOp(name=f"{inst.name}_wsplit{i}")
                        nop.engine = inst.engine
                        nop.sync_info = mybir.SyncInfo(on_wait=[w], on_update=[])
                        out.append(nop)
                    inst.sync_info = mybir.SyncInfo(
                        on_wait=keep, on_update=list(si.on_update)
                    )
                    changed = True
                out.append(inst)
            if changed:
                block.instructions = out


def build_moe_kernel(reps=1):
    nc = bass.Bass(name="moe_expert", enable_partition_id=False)
    xt_d = nc.dram_tensor("xt", [128, KD, C], _BF16, kind="ExternalInput")
    w1_d = nc.dram_tensor("w1t", [MH, 128, KD * 128], _BF16, kind="ExternalInput")
    w2_d = nc.dram_tensor("w2t", [MD, 128, MH * 128], _BF16, kind="ExternalInput")
    b1_d = nc.dram_tensor("b1t", [128, MH], _F32, kind="ExternalInput")
    b2_d = nc.dram_tensor("b2t", [128, MD], _F32, kind="ExternalInput")
    yt_d = nc.dram_tensor("yt", [MD, 128, C], _F32, kind="ExternalOutput")

    with tile.TileContext(nc) as tc:
        for _rep in range(reps):
            _build_body(nc, tc, xt_d, w1_d, w2_d, b1_d, b2_d, yt_d)

    _legalize_waits(nc)
    return nc


def _build_body(nc, tc, xt_d, w1_d, w2_d, b1_d, b2_d, yt_d):
    if True:
        with (
            tc.tile_pool(name="xp", bufs=1) as xpool,
            tc.tile_pool(name="bp", bufs=1) as bpool,
            tc.tile_pool(name="w1p", bufs=3) as w1pool,
            tc.tile_pool(name="hp", bufs=1) as hpool,
            tc.tile_pool(name="w2p", bufs=3) as w2pool,
            tc.tile_pool(name="op", bufs=3) as opool,
            tc.tile_pool(name="ps", bufs=8, space="PSUM") as pspool,
        ):
            xt = xpool.tile([128, KD, C], _BF16)
            nc.sync.dma_start(xt[:], xt_d[:])
            b1 = bpool.tile([128, MH], _F32, tag="b1")
            b2 = bpool.tile([128, MD], _F32, tag="b2")
            nc.sync.dma_start(b1[:], b1_d[:])
            nc.sync.dma_start(b2[:], b2_d[:])

            ht = hpool.tile([128, MH, C], _BF16)

            # phase 1: ht[:, mt, :] = gelu(w1[:, :, mt].T @ x + b1[mt])
            for mt in range(MH):
                w1t = w1pool.tile([128, KD * 128], _BF16)
                nc.sync.dma_start(w1t[:], w1_d[mt])
                for nh in range(NH):
                    ps = pspool.tile([128, CT], _F32)
                    for kt in range(KD):
                        nc.tensor.matmul(
                            ps[:],
                            w1t[:, kt * 128:(kt + 1) * 128],
                            xt[:, kt, nh * CT:(nh + 1) * CT],
                            start=(kt == 0),
                            stop=(kt == KD - 1),
                        )
                    nc.scalar.activation(
                        ht[:, mt, nh * CT:(nh + 1) * CT],
                        ps[:],
                        mybir.ActivationFunctionType.Gelu,
                        bias=b1[:, mt:mt + 1],
                        scale=1.0,
                    )

            # phase 2: yt[mt] = w2[:, :, mt].T @ ht + b2[mt]
            for mt in range(MD):
                w2t = w2pool.tile([128, MH * 128], _BF16)
                nc.sync.dma_start(w2t[:], w2_d[mt])
                ot = opool.tile([128, C], _F32)
                for nh in range(NH):
                    ps = pspool.tile([128, CT], _F32)
                    for kt in range(MH):
                        nc.tensor.matmul(
                            ps[:],
                            w2t[:, kt * 128:(kt + 1) * 128],
                            ht[:, kt, nh * CT:(nh + 1) * CT],
                            start=(kt == 0),
                            stop=(kt == MH - 1),
                        )
                    nc.scalar.activation(
                        ot[:, nh * CT:(nh + 1) * CT],
                        ps[:],
                        mybir.ActivationFunctionType.Identity,
                        bias=b2[:, mt:mt + 1],
                        scale=1.0,
                    )
                nc.sync.dma_start(yt_d[mt], ot[:])


_nc_cache = {}


def _get_nc():
    if "nc" not in _nc_cache:
        _nc_cache["nc"] = build_moe_kernel()
    return _nc_cache["nc"]


def _prepare(x, gate_w, gate_b, w1, b1, w2, b2):
    """Host-side routing: gate, top-1 argmax, aux loss, per-expert gather.

    Returns (in_maps, positions, spill, aux)."""
    x = np.asarray(x, np.float32)
    gate_w = np.asarray(gate_w, np.float32)
    gate_b = np.asarray(gate_b, np.float32)
    w1 = np.asarray(w1, np.float32)
    b1 = np.asarray(b1, np.float32)
    w2 = np.asarray(w2, np.float32)
    b2 = np.asarray(b2, np.float32)

    xf = x.reshape(T, D)
    logits = xf @ gate_w + gate_b                      # [T, E]
    top = np.argmax(logits, axis=-1)                   # [T]

    # aux loss, matching jax.nn.softmax -> mean over batch -> sum(m*m)*E
    l64 = logits.astype(np.float64)
    l64 -= l64.max(axis=-1, keepdims=True)
    p = np.exp(l64)
    p /= p.sum(axis=-1, keepdims=True)
    m = p.reshape(B, S, E).mean(axis=0)
    aux = np.float32((m * m).sum() * E)

    positions = [np.nonzero(top == e)[0] for e in range(E)]
    spill = []                                          # (expert, positions) overflow
    in_maps = []
    for e in range(E):
        pos = positions[e]
        if len(pos) > C:
            spill.append((e, pos[C:]))
            pos = pos[:C]
            positions[e] = pos
        xg = np.zeros((C, D), np.float32)
        xg[: len(pos)] = xf[pos]
        xt = np.ascontiguousarray(
            xg.reshape(C, KD, 128).transpose(2, 1, 0)
        ).astype(ml_dtypes.bfloat16)                    # [128, KD, C]
        w1t = np.ascontiguousarray(
            w1[e].reshape(KD, 128, MH, 128).transpose(2, 1, 0, 3)
        ).reshape(MH, 128, KD * 128).astype(ml_dtypes.bfloat16)
        w2t = np.ascontiguousarray(
            w2[e].reshape(MH, 128, MD, 128).transpose(2, 1, 0, 3)
        ).reshape(MD, 128, MH * 128).astype(ml_dtypes.bfloat16)
        b1t = np.ascontiguousarray(b1[e].reshape(MH, 128).T)   # [128, MH]
        b2t = np.ascontiguousarray(b2[e].reshape(MD, 128).T)   # [128, MD]
        in_maps.append({"xt": xt, "w1t": w1t, "w2t": w2t, "b1t": b1t, "b2t": b2t})

    return in_maps, positions, spill, aux


def kernel(x, gate_w, gate_b, w1, b1, w2, b2):
    in_maps, positions, spill, aux = _prepare(
        x, gate_w, gate_b, w1, b1, w2, b2
    )
    x = np.asarray(x, np.float32)
    w1 = np.asarray(w1, np.float32)
    b1 = np.asarray(b1, np.float32)
    w2 = np.asarray(w2, np.float32)
    b2 = np.asarray(b2, np.float32)
    xf = x.reshape(T, D)

    nc = _get_nc()
    res = bass_utils.run_bass_kernel_spmd(nc, in_maps, core_ids=list(range(8)))

    y = np.zeros((T, D), np.float32)
    for e in range(E):
        pos = positions[e]
        yt = res.results[e]["yt"]                       # [MD, 128, C]
        ye = yt.transpose(2, 0, 1).reshape(C, D)
        y[pos] = ye[: len(pos)]

    # overflow safety net (never triggers for the fixed problem inputs)
    if spill:
        import math
        erf = np.frompyfunc(math.erf, 1, 1)
        for e, pos in spill:
            h = xf[pos] @ w1[e] + b1[e]
            h = 0.5 * h * (1.0 + erf(h / np.sqrt(2.0)).astype(np.float32))
            y[pos] = h @ w2[e] + b2[e]

    return y.reshape(B, S, D), np.array(aux, dtype=np.float32)
